# revision 1
# baseline (speedup 1.0000x reference)
"""Trainium2 Bass kernel for nn_KeywordsLoss.

Computes: KLDivLoss(batchmean) between target = softmax(scatter(alpha at
keyword positions)) and logp = log_softmax(mean_s(logits) with [:,0]=0).

Closed form (per batch row b, V=50257, alpha=0.9):
  K_b   = unique non-zero keyword ids (special ids remapped to 0, excluded)
  k_b   = |K_b|
  D_b   = (V - k_b) + k_b * e^a          (softmax denominator of the target)
  m     = mean_s logits[b],  m[0] = 0
  lse   = log sum_v exp(m)
  loss_b = [lse - log D_b] + a*k_b*e^a/D_b - sum(m)/D_b - (e^a-1)*sum_{K_b}(m)/D_b
  loss  = sum_b loss_b / B

Device does the heavy part (823MB mean-reduction + exp/sum stats); the
keyword set -> (multi-hot, coefficients) preprocessing is tiny host work.

Sharding: data-parallel over B: 2 batch rows per core, 8 cores. Each core
returns its partial loss sum; host adds the 8 scalars and divides by B.

Device layout per batch row: acc[p, j] = sum_s logits[b, s, p*393 + j]
(128*393 = 50304 >= V; the 47-element tail on partition 127 is cleaned
before use). DMA blocks of 8 sequence rows with a 3D access pattern
[[393,128],[V,8],[1,393]]; partition 127 intentionally over-reads into the
next row (garbage lands only in the acc tail, memset after the loop). The
very last row of the shard is split into in-bounds pieces instead.
"""

import sys
from contextlib import ExitStack

import numpy as np

if "/opt/trn_rl_repo" not in sys.path:
    sys.path.insert(0, "/opt/trn_rl_repo")

import concourse.bass as bass
import concourse.bacc as bacc
import concourse.mybir as mybir
import concourse.tile as tile
from concourse.bass_utils import run_bass_kernel_spmd

# Problem constants (hardcoded per the harness contract).
V = 50257
B = 16
S2 = 256
NCORES = 8
BLOC = B // NCORES          # batch rows per core = 2
SEG = 393                   # per-partition vocab segment; 128*393 = 50304
TAIL = 128 * SEG - V        # 47 pad elements on partition 127
VLAST = SEG - TAIL          # 346 valid elements on partition 127
NS = 8                      # sequence rows per DMA block
NB = S2 // NS               # blocks per batch row
ALPHA = 0.9
SPECIAL = (101, 102, 117, 120, 0)

F32 = mybir.dt.float32


XLEN = BLOC * S2 * V + TAIL  # shard is fed flat with TAIL zeros appended


def build_program():
    nc = bacc.Bacc("TRN2", target_bir_lowering=False, debug=False)
    x = nc.declare_dram_parameter("x", [1, XLEN], F32, isOutput=False)
    wh = nc.declare_dram_parameter("wh", [BLOC, 128, SEG], F32, isOutput=False)
    mk = nc.declare_dram_parameter("mk", [128, SEG], F32, isOutput=False)
    cf = nc.declare_dram_parameter("cf", [BLOC, 8], F32, isOutput=False)
    out = nc.declare_dram_parameter("out", [1, 1], F32, isOutput=True)

    AF = mybir.ActivationFunctionType
    ALU = mybir.AluOpType
    AX = mybir.AxisListType

    with tile.TileContext(nc) as tc, ExitStack() as ctx:
        io = ctx.enter_context(tc.tile_pool(name="io", bufs=8))
        accp = ctx.enter_context(tc.tile_pool(name="acc", bufs=1))
        scr = ctx.enter_context(tc.tile_pool(name="scr", bufs=2))
        sml = ctx.enter_context(tc.tile_pool(name="sml", bufs=1))
        psp = ctx.enter_context(
            tc.tile_pool(name="ps", bufs=2, space=bass.MemorySpace.PSUM)
        )

        ones = sml.tile([128, 1], F32, tag="ones")
        nc.vector.memset(ones[:], 1.0)
        mkt = sml.tile([128, SEG], F32, tag="mkt")
        nc.sync.dma_start(mkt[:], mk[:])
        contribs = sml.tile([1, BLOC], F32, tag="contribs")

        for b in range(BLOC):
            accw = accp.tile([128, NS * SEG], F32, tag=f"accw{b}")
            nc.vector.memset(accw[:], 0.0)
            for blk in range(NB):
                r0 = b * S2 + blk * NS
                t = io.tile([128, NS * SEG], F32, tag="io")
                src = bass.AP(x, r0 * V, [[SEG, 128], [V, NS], [1, SEG]])
                nc.sync.dma_start(t[:], src)
                nc.vector.tensor_add(accw[:], accw[:], t[:])
            # Fold the NS-wide accumulator down to one SEG-wide column sum.
            w = NS
            while w > 1:
                h = w // 2
                nc.vector.tensor_add(
                    accw[:, 0 : h * SEG],
                    accw[:, 0 : h * SEG],
                    accw[:, h * SEG : w * SEG],
                )
                w = h
            acc = accw[:, 0:SEG]
            # Mask: zeroes the over-read tail and v=0 (so m[0]=0), ones else.
            nc.vector.tensor_mul(acc, acc, mkt[:])

            stats = sml.tile([128, 3], F32, tag=f"st{b}")
            nc.vector.tensor_reduce(stats[:, 0:1], acc, axis=AX.X, op=ALU.add)
            et = scr.tile([128, SEG], F32, tag="scr")
            # exp(m) with the mean folded into the activation scale; accum_out
            # gives the per-partition row sum in the same instruction. The
            # TAIL pad zeros contribute exp(0)=1 each; corrected at the Ln.
            nc.scalar.activation(
                et[:], acc, AF.Exp, scale=1.0 / S2, accum_out=stats[:, 1:2]
            )
            wt = scr.tile([128, SEG], F32, tag="wt")
            nc.sync.dma_start(wt[:], wh[b])
            st2 = scr.tile([128, SEG], F32, tag="scr")
            nc.vector.tensor_mul(st2[:], wt[:], acc)
            nc.vector.tensor_reduce(stats[:, 2:3], st2[:], axis=AX.X, op=ALU.add)
            # Partition-reduce the three stats with a ones-matmul.
            ps = psp.tile([1, 3], F32, tag=f"ps{b}")
            nc.tensor.matmul(ps[:], ones[:], stats[:], start=True, stop=True)

            cft = sml.tile([1, 8], F32, tag=f"cf{b}")
            nc.sync.dma_start(cft[:], cf[b : b + 1, :])
            s4 = sml.tile([1, 4], F32, tag=f"s4{b}")
            nc.vector.tensor_copy(s4[:, 0:3], ps[:])
            # ln((E - TAIL)/D) via scale/bias APs (per-core values from cf).
            nc.scalar.activation(
                s4[:, 1:2], ps[:, 1:2], AF.Ln, scale=cft[:, 4:5], bias=cft[:, 5:6]
            )
            nc.vector.memset(s4[:, 3:4], 1.0)
            sc4 = sml.tile([1, 4], F32, tag=f"sc4{b}")
            nc.vector.tensor_mul(sc4[:], s4[:], cft[:, 0:4])
            nc.vector.tensor_reduce(
                contribs[:, b : b + 1], sc4[:], axis=AX.X, op=ALU.add
            )
        loss_t = sml.tile([1, 1], F32, tag="loss")
        nc.vector.tensor_reduce(loss_t[:], contribs[:], axis=AX.X, op=ALU.add)
        nc.sync.dma_start(out[:], loss_t[:])
    nc.compile()
    return nc


_NC = None


def _get_program():
    global _NC
    if _NC is None:
        _NC = build_program()
    return _NC


def make_host_inputs(logits, keywords):
    """Host preprocessing: per-row multi-hot keyword mask + loss coefficients."""
    kw = np.asarray(keywords)
    ea = float(np.exp(ALPHA))
    coef = np.zeros((B, 8), np.float32)
    whot = np.zeros((B, 128 * SEG), np.float32)
    for bb in range(B):
        row = kw[bb].astype(np.int64)
        row = np.where(np.isin(row, SPECIAL), 0, row)
        uniq = np.unique(row)
        uniq = uniq[uniq != 0]
        k = len(uniq)
        d = (V - k) + k * ea
        coef[bb, 0] = -1.0 / (S2 * d)          # * A   (A = sum of acc)
        coef[bb, 1] = 1.0                      # * ln((E-TAIL)/D)
        coef[bb, 2] = -(ea - 1.0) / (S2 * d)   # * Wv  (dot(whot, acc))
        coef[bb, 3] = ALPHA * k * ea / d       # constant term
        coef[bb, 4] = 1.0 / d                  # Ln scale
        coef[bb, 5] = -TAIL / d                # Ln bias
        whot[bb, uniq] = 1.0
    mask = np.ones((128, SEG), np.float32)
    mask[0, 0] = 0.0
    mask[127, VLAST:] = 0.0
    return whot.reshape(B, 128, SEG), coef, mask


def pad_shard(shard):
    flat = np.empty((1, XLEN), np.float32)
    flat[0, : XLEN - TAIL] = shard.reshape(-1)
    flat[0, XLEN - TAIL :] = 0.0
    return flat


def kernel(logits, keywords):
    logits = np.ascontiguousarray(np.asarray(logits, dtype=np.float32))
    whot, coef, mask = make_host_inputs(logits, keywords)
    nc = _get_program()
    in_maps = []
    for c in range(NCORES):
        sl = slice(c * BLOC, (c + 1) * BLOC)
        in_maps.append(
            {
                "x": pad_shard(logits[sl]),
                "wh": whot[sl],
                "mk": mask,
                "cf": coef[sl],
            }
        )
    res = run_bass_kernel_spmd(nc, in_maps, list(range(NCORES)))
    total = sum(float(r["out"][0, 0]) for r in res.results)
    return np.float32(total / B)



# revision 2
# speedup vs baseline: 1.1692x; 1.1692x over previous
"""Trainium2 Bass kernel for nn_KeywordsLoss.

Computes: KLDivLoss(batchmean) between target = softmax(scatter(alpha at
keyword positions)) and logp = log_softmax(mean_s(logits) with [:,0]=0).

Closed form (per batch row b, V=50257, alpha=0.9):
  K_b   = unique non-zero keyword ids (special ids remapped to 0, excluded)
  k_b   = |K_b|
  D_b   = (V - k_b) + k_b * e^a          (softmax denominator of the target)
  m     = mean_s logits[b],  m[0] = 0
  lse   = log sum_v exp(m)
  loss_b = [lse - log D_b] + a*k_b*e^a/D_b - sum(m)/D_b - (e^a-1)*sum_{K_b}(m)/D_b
  loss  = sum_b loss_b / B

Sharding: data-parallel over B: 2 batch rows per core, 8 cores. Each core
returns its partial loss sum; host adds the 8 scalars and divides by B.

Device dataflow per batch row (the redesign vs the add-loop baseline):
  1. DMA slabs [128, 4096]: partition p = sequence row (two 128-row
     halves), free dim = vocab columns. Contiguous 16 KB per partition —
     ~10x bigger SDMA packets than the vocab-partition layout, which is
     what lifts per-engine DMA throughput to the HBM roofline. Slab loads
     alternate between the two HWDGE rings (SP + ACT) so transfers overlap.
  2. Sequence-reduce each 512-wide chunk with two ones-matmuls (the two
     seq halves) accumulating into PSUM [1, 512] — Tensor engine does the
     entire 256-row sum; the Vector engine only evacuates PSUM to SBUF.
  3. Staged slab sums are DMA'd to an HBM scratch row ms[b] (50688 = 128*396
     elements, tail zeroed), then reloaded once as [128, 396] to restore a
     full-lane vocab-partition layout for the cheap stats tail
     (mask, sum, exp-accum, keyword dot, ones-matmul partition reduce).
All ms writes and the reload are issued on the GPSIMD (SWDGE) queue so
the two HWDGE rings carry nothing but the big slab loads back-to-back;
the ms RAW hazard is covered by Tile's shadow-memory dependency tracking
plus SWDGE same-queue FIFO order.
"""

import sys
from contextlib import ExitStack

import numpy as np

if "/opt/trn_rl_repo" not in sys.path:
    sys.path.insert(0, "/opt/trn_rl_repo")

import concourse.bass as bass
import concourse.bacc as bacc
import concourse.mybir as mybir
import concourse.tile as tile
from concourse.bass_utils import run_bass_kernel_spmd

# Problem constants (hardcoded per the harness contract).
V = 50257
B = 16
S2 = 256
NCORES = 8
BLOC = B // NCORES          # batch rows per core = 2
F = 4096                    # vocab slab width per DMA tile
NSLAB = (V + F - 1) // F    # 13 slabs (12 full + 1105-wide tail)
CH = 512                    # matmul moving-operand chunk (fp32 max)
SEG2 = 396                  # reload layout: 128 * 396 = 50688
MS = 128 * SEG2             # padded mean-vector length
TAIL2 = MS - V              # 431 zero pad elements
LAST_W = V - (NSLAB - 1) * F        # 1105 valid cols in last slab
LAST_STG = MS - (NSLAB - 1) * F     # 1536 staged cols in last slab
ALPHA = 0.9
SPECIAL = (101, 102, 117, 120, 0)

F32 = mybir.dt.float32
F32R = mybir.dt.float32r

XLEN = BLOC * S2 * V


def build_program():
    nc = bacc.Bacc("TRN2", target_bir_lowering=False, debug=False)
    x = nc.declare_dram_parameter("x", [1, XLEN], F32, isOutput=False)
    wh = nc.declare_dram_parameter("wh", [BLOC, 128, SEG2], F32, isOutput=False)
    mk = nc.declare_dram_parameter("mk", [128, SEG2], F32, isOutput=False)
    cf = nc.declare_dram_parameter("cf", [BLOC, 8], F32, isOutput=False)
    out = nc.declare_dram_parameter("out", [1, 1], F32, isOutput=True)
    ms = nc.declare_dram_parameter("ms", [1, BLOC * MS], F32, isOutput=True)

    AF = mybir.ActivationFunctionType
    ALU = mybir.AluOpType
    AX = mybir.AxisListType

    with tile.TileContext(nc) as tc, ExitStack() as ctx:
        io = ctx.enter_context(tc.tile_pool(name="io", bufs=5))
        stg = ctx.enter_context(tc.tile_pool(name="stg", bufs=2))
        scr = ctx.enter_context(tc.tile_pool(name="scr", bufs=2))
        sml = ctx.enter_context(tc.tile_pool(name="sml", bufs=1))
        psp = ctx.enter_context(
            tc.tile_pool(name="ps", bufs=7, space=bass.MemorySpace.PSUM)
        )
        psp3 = ctx.enter_context(
            tc.tile_pool(name="ps3", bufs=1, space=bass.MemorySpace.PSUM)
        )

        ones = sml.tile([128, 1], F32, tag="ones")
        nc.vector.memset(ones[:], 1.0)
        # fp32r twin of `ones` for the big matmuls: the BIR verifier
        # requires every producer feeding an fp32r matmul to emit fp32r,
        # and memset cannot (copy-cast can).
        onesr = sml.tile([128, 1], F32, tag="onesr")
        nc.vector.tensor_copy(onesr[:].bitcast(F32R), ones[:])
        zcol = sml.tile([128, 1], F32, tag="zcol")
        nc.vector.memset(zcol[:], 0.0)
        mkt = sml.tile([128, SEG2], F32, tag="mkt")
        nc.gpsimd.dma_start(mkt[:], mk[:])
        contribs = sml.tile([1, BLOC], F32, tag="contribs")

        mrows = []
        stats = []
        ring = 0
        for b in range(BLOC):
            for t in range(NSLAB):
                c0 = t * F
                w = F if t < NSLAB - 1 else LAST_W
                tt = io.tile([128, 2 * F], F32, tag="io")
                eng = nc.sync if ring % 2 == 0 else nc.scalar
                ring += 1
                if t < NSLAB - 1:
                    # One DMA covers both 128-row sequence halves of the
                    # slab: 2 blocks of w contiguous cols at stride 128*V.
                    src = bass.AP(
                        x, (b * S2) * V + c0, [[V, 128], [128 * V, 2], [1, w]]
                    )
                    eng.dma_start(tt[:].bitcast(F32R), src.bitcast(F32R))
                else:
                    for h in range(2):
                        src = bass.AP(
                            x, (b * S2 + h * 128) * V + c0, [[V, 128], [1, w]]
                        )
                        eng.dma_start(
                            tt[:, h * F : h * F + w].bitcast(F32R),
                            src.bitcast(F32R),
                        )
                        # fp32r matmuls need an even moving free dim; widen
                        # the 81-wide tail chunk to 82 over a zeroed column.
                        nc.vector.tensor_copy(
                            tt[:, h * F + w : h * F + w + 1].bitcast(F32R),
                            zcol[:],
                        )
                halves = [tt[:, 0:F], tt[:, F : 2 * F]]
                wlen = F if t < NSLAB - 1 else LAST_STG
                st = stg.tile([1, F], F32, tag="stg")
                if t == NSLAB - 1:
                    nc.vector.memset(st[:, w:wlen], 0.0)
                we = w if w % 2 == 0 else w + 1
                nch = (we + CH - 1) // CH
                for j in range(nch):
                    cw = min(CH, we - j * CH)
                    ps = psp.tile([1, CH], F32, tag="ps")
                    # fp32r runs the PE at 1 cycle/row for N>=256 (plain
                    # fp32 needs 2 half-rate passes = 4x the cycles).
                    nc.tensor.matmul(
                        ps[:, :cw],
                        onesr[:].bitcast(F32R),
                        halves[0][:, j * CH : j * CH + cw].bitcast(F32R),
                        start=True,
                        stop=False,
                    )
                    nc.tensor.matmul(
                        ps[:, :cw],
                        onesr[:].bitcast(F32R),
                        halves[1][:, j * CH : j * CH + cw].bitcast(F32R),
                        start=False,
                        stop=True,
                    )
                    nc.vector.tensor_copy(st[:, j * CH : j * CH + cw], ps[:, :cw])
                dst = bass.AP(ms, b * MS + c0, [[1, 1], [1, wlen]])
                nc.gpsimd.dma_start(dst, st[:, :wlen])

            # Row stats phase A (everything except exp/Ln, which need ACT —
            # deferred to the end so the ACT HWDGE ring keeps streaming).
            mrow = sml.tile([128, SEG2], F32, tag=f"mrow{b}")
            nc.gpsimd.dma_start(mrow[:], bass.AP(ms, b * MS, [[SEG2, 128], [1, SEG2]]))
            nc.vector.tensor_mul(mrow[:], mrow[:], mkt[:])
            stt = sml.tile([128, 3], F32, tag=f"st{b}")
            nc.vector.tensor_reduce(stt[:, 0:1], mrow[:], axis=AX.X, op=ALU.add)
            wt = scr.tile([128, SEG2], F32, tag="wt")
            nc.gpsimd.dma_start(wt[:], wh[b])
            st2 = scr.tile([128, SEG2], F32, tag="scr")
            nc.vector.tensor_mul(st2[:], wt[:], mrow[:])
            nc.vector.tensor_reduce(stt[:, 2:3], st2[:], axis=AX.X, op=ALU.add)
            mrows.append(mrow)
            stats.append(stt)

        for b in range(BLOC):
            et = scr.tile([128, SEG2], F32, tag="scr")
            nc.scalar.activation(
                et[:], mrows[b][:], AF.Exp, scale=1.0 / S2,
                accum_out=stats[b][:, 1:2],
            )
            ps3 = psp3.tile([1, 3], F32, tag="ps3")
            nc.tensor.matmul(ps3[:], ones[:], stats[b][:], start=True, stop=True)
            cft = sml.tile([1, 8], F32, tag=f"cf{b}")
            nc.gpsimd.dma_start(cft[:], cf[b : b + 1, :])
            s4 = sml.tile([1, 4], F32, tag=f"s4{b}")
            nc.vector.tensor_copy(s4[:, 0:3], ps3[:])
            # ln((E - TAIL2)/D) via scale/bias APs (per-row values from cf).
            nc.scalar.activation(
                s4[:, 1:2], ps3[:, 1:2], AF.Ln, scale=cft[:, 4:5], bias=cft[:, 5:6]
            )
            nc.vector.memset(s4[:, 3:4], 1.0)
            sc4 = sml.tile([1, 4], F32, tag=f"sc4{b}")
            nc.vector.tensor_mul(sc4[:], s4[:], cft[:, 0:4])
            nc.vector.tensor_reduce(
                contribs[:, b : b + 1], sc4[:], axis=AX.X, op=ALU.add
            )
        loss_t = sml.tile([1, 1], F32, tag="loss")
        nc.vector.tensor_reduce(loss_t[:], contribs[:], axis=AX.X, op=ALU.add)
        nc.gpsimd.dma_start(out[:], loss_t[:])
    nc.compile()
    return nc


_NC = None


def _get_program():
    global _NC
    if _NC is None:
        _NC = build_program()
    return _NC


def make_host_inputs(logits, keywords):
    """Host preprocessing: per-row multi-hot keyword mask + loss coefficients."""
    kw = np.asarray(keywords)
    ea = float(np.exp(ALPHA))
    coef = np.zeros((B, 8), np.float32)
    whot = np.zeros((B, MS), np.float32)
    for bb in range(B):
        row = kw[bb].astype(np.int64)
        row = np.where(np.isin(row, SPECIAL), 0, row)
        uniq = np.unique(row)
        uniq = uniq[uniq != 0]
        k = len(uniq)
        d = (V - k) + k * ea
        coef[bb, 0] = -1.0 / (S2 * d)          # * A   (A = sum of acc)
        coef[bb, 1] = 1.0                      # * ln((E-TAIL2)/D)
        coef[bb, 2] = -(ea - 1.0) / (S2 * d)   # * Wv  (dot(whot, acc))
        coef[bb, 3] = ALPHA * k * ea / d       # constant term
        coef[bb, 4] = 1.0 / d                  # Ln scale
        coef[bb, 5] = -TAIL2 / d               # Ln bias
        whot[bb, uniq] = 1.0
    # Mask: zero v=0 (so m[0]=0); the MS-V tail is already written as
    # explicit zeros on device, but zero it in the mask too for safety.
    mask = np.ones((128, SEG2), np.float32)
    mask[0, 0] = 0.0
    flat = mask.reshape(-1)
    flat[V:] = 0.0
    return whot.reshape(B, 128, SEG2), coef, mask


def make_in_maps(inputs):
    logits = np.ascontiguousarray(np.asarray(inputs["logits"], dtype=np.float32))
    whot, coef, mask = make_host_inputs(logits, inputs["keywords"])
    in_maps = []
    for c in range(NCORES):
        sl = slice(c * BLOC, (c + 1) * BLOC)
        in_maps.append(
            {
                "x": logits[sl].reshape(1, XLEN),
                "wh": whot[sl],
                "mk": mask,
                "cf": coef[sl],
            }
        )
    return in_maps


def reduce_results(results):
    total = sum(float(r["out"][0, 0]) for r in results)
    return total / B


def kernel(logits, keywords):
    nc = _get_program()
    in_maps = make_in_maps({"logits": logits, "keywords": keywords})
    res = run_bass_kernel_spmd(nc, in_maps, list(range(NCORES)))
    return np.float32(reduce_results(res.results))
